# revision 30
# baseline (speedup 1.0000x reference)
"""Causal attention (B=4, S=2048, D=1024, single 1024-dim head) on 8 TRN2 cores.

Sharding: data-parallel over batch (4) x 2-way causal-balanced query split
(zigzag 128-row query blocks; core role 0 gets global blocks
{0,3,4,7,8,11,12,15}, role 1 {1,2,5,6,9,10,13,14}).  Every core runs the same
SPMD program over 8 query slots with k-chunk counts {16,14,12,10,8,6,4,2};
causality differences between the cores are expressed purely through per-core
input data (gathered q columns + mask tiles), never through the instruction
stream.

Key algebraic trick: the q-projection is absorbed into the k-projection on the
host.  scores = (x_q W_q^T)(x_k W_k^T)^T = x_q (W_q^T W_k) x_k^T, so with
A := W_q^T W_k precomputed on the host (fp32), the device only computes
  tT = (A^T)^T x^T   [din_i, 2048]   (one projection instead of two)
  per q-slot (128 cols), per k-chunk (128 rows):
      sT   = t_chunk^T xq_slot  [128k, 128q]   (PSUM, 8 i-chunk matmuls)
      expT = exp(sT/32)         (ACT, PSUM->SBUF fp16; scores are O(+-8) so
                                 no max-subtraction is needed)
      mask-multiply (DVE) for the last 2 chunks of the slot (host tiles:
      ones / triangular / zero as causality requires)
      dacc += expT              (DVE fp32 partial sums; one fp32 ones-matmul
                                 per slot reduces over partitions afterwards)
      ctx[d] += xn_chunk[:,d]^T expT   (PSUM accumulate: ctx = attn @ x,
                                 using attn@x@WvT == attn@(x WvT) assoc.)
  after every slot pair: outT = (WvT^T ctx) * (1/denominator)  [dout, 256]
  (out-projection interleaved into phase 2 so the PE never waits on a
  separate phase 3).
Matmul operands are fp16 (host-converted); accumulation PSUM is fp32, the
softmax denominator path is fp32, output is fp32.
"""

import os
import sys

sys.path.insert(0, "/opt/trn_rl_repo")

import numpy as np

B, S, DIN, DOUT = 4, 2048, 1024, 1024
P = 128
NQ = 1024  # q rows per core
ND = DIN // P
NO = DOUT // P
NK = S // P  # 16 key chunks
NS = 8  # q slots per core (128 rows each)
NCORES = 8
# global 128-row q-block per (core-half, slot); slot s has L[s] k-chunks
BLK = [[15, 12, 11, 8, 7, 4, 3, 0], [14, 13, 10, 9, 6, 5, 2, 1]]
L = [16, 14, 12, 10, 8, 6, 4, 2]  # k-chunks per slot (uniform across cores)

_NC_CACHE = {}


def _build_nc():
    import concourse.mybir as mybir
    import concourse.tile as tile
    from concourse import bacc
    from contextlib import ExitStack

    f32 = mybir.dt.float32
    f16 = mybir.dt.float16
    EXP = mybir.ActivationFunctionType.Exp

    nc = bacc.Bacc("TRN2", target_bir_lowering=False, debug=False,
                   num_devices=NCORES)

    xqT_d = nc.dram_tensor("xqT", [DIN, NQ], f16, kind="ExternalInput").ap()
    xT_d = nc.dram_tensor("xT", [DIN, S], f16, kind="ExternalInput").ap()
    xn_d = nc.dram_tensor("xn", [S, DIN], f16, kind="ExternalInput").ap()
    aT_d = nc.dram_tensor("aT", [DIN, DIN], f16, kind="ExternalInput").ap()
    wvT_d = nc.dram_tensor("wvT", [DIN, DOUT], f16, kind="ExternalInput").ap()
    masks_d = nc.dram_tensor("masks", [P, 16 * P], f16, kind="ExternalInput").ap()
    ones_d = nc.dram_tensor("ones", [P, 160], f16, kind="ExternalInput").ap()
    outT_d = nc.dram_tensor("outT", [DOUT, NQ], f32, kind="ExternalOutput").ap()

    with tile.TileContext(nc) as tc:
        with ExitStack() as es:
            tT_pool = es.enter_context(tc.tile_pool(name="tTp", bufs=1))
            xq_pool = es.enter_context(tc.tile_pool(name="xqp", bufs=1))
            ctx_pool = es.enter_context(tc.tile_pool(name="ctxp", bufs=1))
            cst_pool = es.enter_context(tc.tile_pool(name="cst", bufs=1))
            xn_pool = es.enter_context(tc.tile_pool(name="xnp", bufs=1))
            wv_pool = es.enter_context(tc.tile_pool(name="wvp", bufs=1))

            # per-half tiles: score chunks 0-7 only depend on the half-0
            # evacuations (whole-tile dependency granularity would otherwise
            # stall slot 0 on the very last projection evacuation)
            tT = [[tT_pool.tile([P, 1024], f16, name=f"tT{o}_{h}",
                                tag=f"tT{o}_{h}") for h in range(2)]
                  for o in range(NO)]
            xqs = [xq_pool.tile([P, NQ], f16, name=f"xq{d}", tag=f"xq{d}")
                   for d in range(ND)]
            zeroT = cst_pool.tile([P, 512], f16, name="zeroT", tag="zeroT")
            nc.vector.memset(zeroT[:], 0.0)
            onesT = cst_pool.tile([P, 160], f16, name="onesT", tag="onesT")
            ones_row = onesT[0:1, 32:160]  # [1, 128]
            ones32 = cst_pool.tile([P, 1], f32, name="ones32", tag="ones32")
            nc.vector.memset(ones32[:], 1.0)
            # x rows (AV stationary operand): resident for all of phase 2
            xn16 = [xn_pool.tile([P, DIN], f16, name=f"xn{c}", tag=f"xn{c}")
                    for c in range(NK)]

            # ---------------- phase 1: t-projection ----------------
            with tc.tile_pool(name="xs", bufs=16) as x_pool, \
                 tc.tile_pool(name="ws", bufs=8) as w_pool, \
                 tc.tile_pool(name="pps", bufs=5, space="PSUM") as proj_ps:
                # PE warmup during the initial DMA head: harmless matmuls on
                # the (memset, DMA-free) zero tile keep the HAM clock gate
                # from idling while the first x/A tiles stream in.
                wu = proj_ps.tile([P, 128], f32, name="wu", tag="wu", bufs=1)
                for r in range(48):
                    nc.tensor.matmul(wu[:], zeroT[:, 0:128], zeroT[:, 0:128],
                                     start=True, stop=True,
                                     skip_group_check=True)

                # tT = A^T @ x^T, all 2048 keys in two column halves.
                # d-outer loop with 2 concurrent PSUM chains -> each weight
                # slice (lhsT) is loaded once per 2 matmuls.
                # interleave the aT/xh0 loads d-wise so the first t-proj
                # accumulation chain can trickle-start as operand pairs land
                # instead of waiting for the whole 4MB group
                ats = []
                xhs_all = {}
                for d in range(ND):
                    at = w_pool.tile([P, DIN], f16, name=f"at{d}", tag="ws")
                    nc.sync.dma_start(at[:], aT_d[d * P:(d + 1) * P, :])
                    ats.append(at)
                    xh = x_pool.tile([P, 1024], f16, name=f"xh0_{d}", tag="xs")
                    nc.sync.dma_start(xh[:], xT_d[d * P:(d + 1) * P, 0:1024])
                    xhs_all[(0, d)] = xh
                for d in range(ND):
                    xh = x_pool.tile([P, 1024], f16, name=f"xh1_{d}", tag="xs")
                    nc.sync.dma_start(
                        xh[:], xT_d[d * P:(d + 1) * P, 1024:2048])
                    xhs_all[(1, d)] = xh
                # score rhs operand (x^T gathered q cols) prefetches right
                # behind the t-proj operands (own pool slots -> no WAR
                # serialization)
                for d in range(ND):
                    nc.sync.dma_start(xqs[d][:], xqT_d[d * P:(d + 1) * P, :])
                # attention stationary x rows stream in behind everything;
                # the tiny constants (needed only mid-phase-2) come last
                for c in range(NK):
                    nc.sync.dma_start(xn16[c][:], xn_d[c * P:(c + 1) * P, :])
                nc.sync.dma_start(onesT[:], ones_d[:])

                for half in range(2):
                    xhs = [xhs_all[(half, d)] for d in range(ND)]
                    for o in range(NO):
                        pos = [proj_ps.tile([P, 512], f32, name=f"pok{kp}",
                                            tag="po") for kp in range(2)]
                        for d in range(ND):
                            for kp in range(2):
                                nc.tensor.matmul(
                                    pos[kp][:],
                                    ats[d][:, o * P:(o + 1) * P],
                                    xhs[d][:, kp * 512:(kp + 1) * 512],
                                    start=(d == 0), stop=(d == ND - 1))
                        for kp in range(2):
                            col = kp * 512
                            nc.vector.tensor_copy(
                                tT[o][half][:, col:col + 512], pos[kp][:])

            # ------- phase 2: attention + interleaved out-projection -------
            with tc.tile_pool(name="exq", bufs=5) as exp_pool, \
                 tc.tile_pool(name="dac", bufs=2) as dacc_pool, \
                 tc.tile_pool(name="obp", bufs=4) as out_pool, \
                 tc.tile_pool(name="sps", bufs=2, space="PSUM") as sT_ps, \
                 tc.tile_pool(name="cps", bufs=4, space="PSUM") as ctx_ps, \
                 tc.tile_pool(name="ops", bufs=2, space="PSUM") as out_ps:
                maskT = cst_pool.tile([P, 16 * P], f16, name="maskT",
                                      tag="maskT")
                nc.sync.dma_start(maskT[:], masks_d[:])
                wvs = []
                for d in range(ND):
                    wv = wv_pool.tile([P, DOUT], f16, name=f"wv{d}",
                                      tag=f"wv{d}")
                    nc.sync.dma_start(wv[:], wvT_d[d * P:(d + 1) * P, :])
                    wvs.append(wv)

                # per-pair tiles: ctx [128,256] per d (slot s -> column
                # half s%2), reciprocal [1,256]
                ctxP = {}
                recP = {}
                for p in range(NS // 2):
                    for d in range(ND):
                        ctxP[(d, p)] = ctx_pool.tile(
                            [P, 256], f16, name=f"ctx{d}_{p}",
                            tag=f"ctx{d}_{p}")
                    recP[p] = cst_pool.tile([1, 256], f16, name=f"rec{p}",
                                            tag=f"rec{p}")

                def do_outproj(p):
                    # outT[:, p*256:(p+1)*256] = (Wv ctx^T) * (1/denominator)
                    bc = out_ps.tile([P, 256], f32, name="bc", tag="poo")
                    nc.tensor.matmul(bc[:], ones_row, recP[p][:],
                                     start=True, stop=True)
                    bcs = out_pool.tile([P, 256], f32, name="bcs", tag="bcs",
                                        bufs=2)
                    nc.vector.tensor_copy(bcs[:], bc[:])
                    for o in range(NO):
                        po = out_ps.tile([P, 256], f32, name="poo", tag="poo")
                        for d in range(ND):
                            nc.tensor.matmul(
                                po[:],
                                wvs[d][:, o * P:(o + 1) * P],
                                ctxP[(d, p)][:],
                                start=(d == 0), stop=(d == ND - 1))
                        ob = out_pool.tile([P, 256], f32, name="ob", tag="ob",
                                           bufs=2)
                        nc.vector.tensor_mul(ob[:], po[:], bcs[:])
                        nc.sync.dma_start(
                            outT_d[o * P:(o + 1) * P, p * 256:(p + 1) * 256],
                            ob[:])

                daccP = [None]
                for s in range(NS):
                    q0 = s * P
                    ls = L[s]
                    cps = [ctx_ps.tile([P, 512], f32, name=f"cps{s}_{i}",
                                       tag="cps") for i in range(2)]
                    # denominator partials for the whole slot pair live in
                    # one [128,256] tile (slot -> column half) so the pair
                    # needs a single partition-reduce matmul + reciprocal
                    if s % 2 == 0:
                        daccP[0] = dacc_pool.tile([P, 256], f32,
                                                  name=f"dacc{s // 2}",
                                                  tag="dacc")
                    dacc = daccP[0][:, (s % 2) * P:(s % 2) * P + P]

                    def st_chunk(c):
                        st = sT_ps.tile([P, P], f32, name="st", tag="st")
                        for o in range(NO):
                            nc.tensor.matmul(
                                st[:],
                                tT[o][c // 8][:, (c % 8) * P:(c % 8 + 1) * P],
                                xqs[o][:, q0:q0 + P],
                                start=(o == 0), stop=(o == NO - 1))
                        et = exp_pool.tile([P, P], f16, name="et", tag="et")
                        nc.scalar.activation(et[:], st[:], EXP, scale=1.0 / 32.0)
                        if c >= ls - 2:
                            m = 2 * s + (c - (ls - 2))
                            et2 = exp_pool.tile([P, P], f16, name="et2",
                                                tag="et2")
                            nc.vector.tensor_mul(
                                et2[:], et[:], maskT[:, m * P:(m + 1) * P])
                            et = et2
                        return et

                    def av_chunk(c, et):
                        # softmax denominator partials accumulate on the
                        # (otherwise idle) DVE instead of spending PE
                        # matmuls; one fp32 ones-matmul per slot reduces over
                        # partitions afterwards
                        if c == 0:
                            nc.vector.tensor_copy(dacc, et[:])
                        else:
                            nc.vector.tensor_add(dacc, dacc, et[:])
                        for d in range(ND):
                            acc = cps[d // 4][:, (d % 4) * P:(d % 4) * P + P]
                            nc.tensor.matmul(
                                acc, xn16[c][:, d * P:(d + 1) * P], et[:],
                                start=False, stop=(c == ls - 1),
                                skip_group_check=True)

                    # software pipeline: score chains run 2 chunks ahead of
                    # the AV matmuls.  Zero the ctx banks with DVE memsets
                    # instead of dummy matmuls: with the data zeroed, a
                    # start=False matmul is correct for ANY has_written state
                    # (set bit -> accumulate onto 0; clear bit -> plain
                    # overwrite that sets the bit), and the PE spends nothing
                    # on initialization.
                    ets = {0: st_chunk(0)}
                    if ls > 1:
                        ets[1] = st_chunk(1)
                    for i in range(2):
                        nc.vector.memset(cps[i][:], 0.0)
                    # the previous pair's out-projection slots in here: its
                    # reciprocal (DVE, queued at the pair boundary) has
                    # drained behind the two score chains above, so the bc
                    # broadcast matmul never stalls the PE queue
                    if s >= 2 and s % 2 == 0:
                        do_outproj(s // 2 - 1)
                    for c in range(ls):
                        if c + 2 < ls:
                            ets[c + 2] = st_chunk(c + 2)
                        av_chunk(c, ets.pop(c))

                    # evacuate ctx accumulators into the per-pair tiles
                    # (frees the PSUM banks without waiting on the
                    # denominator chain).  Alternate evacuations onto the
                    # otherwise-idle scalar engine to unload the DVE.
                    # all evacuations on the scalar engine: its queue stays
                    # short, and the DVE queue is then empty at the pair
                    # boundary when the reciprocal / bcs / ob chain needs it
                    for d in range(ND):
                        srcp = cps[d // 4][:, (d % 4) * P:(d % 4) * P + P]
                        dst = ctxP[(d, s // 2)][:, (s % 2) * P:(s % 2) * P + P]
                        nc.scalar.copy(dst, srcp)
                    # at the pair boundary: partition-reduce both slots'
                    # denominator partials (their DVE chains finished long
                    # ago, so the PE never stalls here), then reciprocal
                    # into the pair's [1,256] tile.  The out-projection
                    # itself is deferred into the next slot's pipeline.
                    if s % 2 == 1:
                        # dsum lives in the out_ps rotation: its reciprocal
                        # (scalar engine, short queue) drains while the next
                        # slot's score chains run, so neither the next po
                        # chain nor st chain ever waits on it
                        dsum = out_ps.tile([1, 256], f32, name=f"dsum{s // 2}",
                                           tag="poo")
                        nc.tensor.matmul(dsum[:], ones32[:], daccP[0][:],
                                         start=True, stop=True)
                        with nc.allow_low_precision(
                                reason="fp16 recip feeds fp16 bcast"):
                            nc.vector.reciprocal(recP[s // 2][:], dsum[:])
                do_outproj(NS // 2 - 1)

    nc.compile()
    return nc


def _get_nc():
    if "nc" not in _NC_CACHE:
        _NC_CACHE["nc"] = _build_nc()
    return _NC_CACHE["nc"]


def _make_masks(h):
    """[128, 16*128] mask tiles: slot s, last-2 chunks; 1.0 where key
    128c+p <= query 128*BLK[h][s]+j (all-ones / triangular / all-zero)."""
    mk = np.zeros((P, 16 * P), dtype=np.float16)
    p = np.arange(P)[:, None]
    j = np.arange(P)[None, :]
    for s in range(NS):
        b = BLK[h][s]
        for m in range(2):
            c = L[s] - 2 + m
            mk[:, (2 * s + m) * P:(2 * s + m + 1) * P] = (
                (P * c + p) <= (P * b + j)).astype(np.float16)
    return mk


def kernel(x, W_q, W_k, W_v):
    from concourse.bass_utils import run_bass_kernel_spmd

    x = np.asarray(x, dtype=np.float32)
    x16 = x.astype(np.float16)
    wq = np.asarray(W_q, dtype=np.float32)
    wk = np.asarray(W_k, dtype=np.float32)
    # A^T = W_k^T W_q  (scores = x_q (W_q^T W_k) x_k^T; tT = (A^T)^T x^T)
    aT = np.ascontiguousarray((wk.T @ wq).astype(np.float16))
    wvT = np.ascontiguousarray(np.asarray(W_v, dtype=np.float32).T
                               .astype(np.float16))

    ones = np.zeros((P, 160), dtype=np.float16)
    ones[:, 0] = 1.0
    ones[0, 32:160] = 1.0
    masks_h = [_make_masks(0), _make_masks(1)]

    in_maps = []
    for b in range(B):
        xTb = np.ascontiguousarray(x16[b].T)
        for h in range(2):
            qcols = np.concatenate(
                [np.arange(g * P, (g + 1) * P) for g in BLK[h]])
            in_maps.append(dict(
                xqT=np.ascontiguousarray(xTb[:, qcols]),
                xT=xTb,
                xn=np.ascontiguousarray(x16[b]),
                aT=aT, wvT=wvT,
                masks=masks_h[h],
                ones=ones,
            ))

    nc = _get_nc()
    res = run_bass_kernel_spmd(nc, in_maps, core_ids=list(range(NCORES)),
                               trace=bool(os.environ.get("KERNEL_TRACE")))
    if os.environ.get("KERNEL_TRACE"):
        _NC_CACHE["last_results"] = res

    out = np.empty((B, S, DOUT), dtype=np.float32)
    for b in range(B):
        for h in range(2):
            oT = res.results[b * 2 + h]["outT"]
            for s2, g in enumerate(BLK[h]):
                out[b, g * P:(g + 1) * P, :] = \
                    oT[:, s2 * P:(s2 + 1) * P].T
    return out


# revision 32
# speedup vs baseline: 1.0373x; 1.0373x over previous
"""Causal attention (B=4, S=2048, D=1024, single 1024-dim head) on 8 TRN2 cores.

Sharding: data-parallel over batch (4) x 2-way causal-balanced query split
(zigzag 128-row query blocks; core role 0 gets global blocks
{0,3,4,7,8,11,12,15}, role 1 {1,2,5,6,9,10,13,14}).  Every core runs the same
SPMD program over 8 query slots with k-chunk counts {16,14,12,10,8,6,4,2};
causality differences between the cores are expressed purely through per-core
input data (gathered q columns + mask tiles), never through the instruction
stream.

Key algebraic trick: the q-projection is absorbed into the k-projection on the
host.  scores = (x_q W_q^T)(x_k W_k^T)^T = x_q (W_q^T W_k) x_k^T, so with
A := W_q^T W_k precomputed on the host (fp32), the device only computes
  tT = (A^T)^T x^T   [din_i, 2048]   (one projection instead of two)
  per q-slot (128 cols), per k-chunk (128 rows):
      sT   = t_chunk^T xq_slot  [128k, 128q]   (PSUM, 8 i-chunk matmuls)
      expT = exp(sT/32)         (ACT, PSUM->SBUF fp16; scores are O(+-8) so
                                 no max-subtraction is needed)
      mask-multiply (DVE) for the last 2 chunks of the slot (host tiles:
      ones / triangular / zero as causality requires)
      dacc += expT              (DVE fp32 partial sums; one fp32 ones-matmul
                                 per slot reduces over partitions afterwards)
      ctx[d] += xn_chunk[:,d]^T expT   (PSUM accumulate: ctx = attn @ x,
                                 using attn@x@WvT == attn@(x WvT) assoc.)
  after every slot pair: outT = (WvT^T ctx) * (1/denominator)  [dout, 256]
  (out-projection interleaved into phase 2 so the PE never waits on a
  separate phase 3).
Matmul operands are fp16 (host-converted); accumulation PSUM is fp32, the
softmax denominator path is fp32, output is fp32.
"""

import os
import sys

sys.path.insert(0, "/opt/trn_rl_repo")

import numpy as np

B, S, DIN, DOUT = 4, 2048, 1024, 1024
P = 128
NQ = 1024  # q rows per core
ND = DIN // P
NO = DOUT // P
NK = S // P  # 16 key chunks
NS = 8  # q slots per core (128 rows each)
NCORES = 8
# global 128-row q-block per (core-half, slot); slot s has L[s] k-chunks
BLK = [[15, 12, 11, 8, 7, 4, 3, 0], [14, 13, 10, 9, 6, 5, 2, 1]]
L = [16, 14, 12, 10, 8, 6, 4, 2]  # k-chunks per slot (uniform across cores)

_NC_CACHE = {}


def _build_nc():
    import concourse.mybir as mybir
    import concourse.tile as tile
    from concourse import bacc
    from contextlib import ExitStack

    f32 = mybir.dt.float32
    f16 = mybir.dt.float16
    EXP = mybir.ActivationFunctionType.Exp

    nc = bacc.Bacc("TRN2", target_bir_lowering=False, debug=False,
                   num_devices=NCORES)

    xqT_d = nc.dram_tensor("xqT", [DIN, NQ], f16, kind="ExternalInput").ap()
    xT_d = nc.dram_tensor("xT", [DIN, S], f16, kind="ExternalInput").ap()
    xn_d = nc.dram_tensor("xn", [S, DIN], f16, kind="ExternalInput").ap()
    aT_d = nc.dram_tensor("aT", [DIN, DIN], f16, kind="ExternalInput").ap()
    wvT_d = nc.dram_tensor("wvT", [DIN, DOUT], f16, kind="ExternalInput").ap()
    masks_d = nc.dram_tensor("masks", [P, 16 * P], f16, kind="ExternalInput").ap()
    ones_d = nc.dram_tensor("ones", [P, 160], f16, kind="ExternalInput").ap()
    outT_d = nc.dram_tensor("outT", [DOUT, NQ], f32, kind="ExternalOutput").ap()

    with tile.TileContext(nc) as tc:
        with ExitStack() as es:
            tT_pool = es.enter_context(tc.tile_pool(name="tTp", bufs=1))
            xq_pool = es.enter_context(tc.tile_pool(name="xqp", bufs=1))
            ctx_pool = es.enter_context(tc.tile_pool(name="ctxp", bufs=1))
            cst_pool = es.enter_context(tc.tile_pool(name="cst", bufs=1))
            xn_pool = es.enter_context(tc.tile_pool(name="xnp", bufs=1))
            wv_pool = es.enter_context(tc.tile_pool(name="wvp", bufs=1))

            # per-half tiles: score chunks 0-7 only depend on the half-0
            # evacuations (whole-tile dependency granularity would otherwise
            # stall slot 0 on the very last projection evacuation)
            tT = [[tT_pool.tile([P, 1024], f16, name=f"tT{o}_{h}",
                                tag=f"tT{o}_{h}") for h in range(2)]
                  for o in range(NO)]
            xqs = [xq_pool.tile([P, NQ], f16, name=f"xq{d}", tag=f"xq{d}")
                   for d in range(ND)]
            zeroT = cst_pool.tile([P, 512], f16, name="zeroT", tag="zeroT")
            nc.vector.memset(zeroT[:], 0.0)
            onesT = cst_pool.tile([P, 160], f16, name="onesT", tag="onesT")
            ones_row = onesT[0:1, 32:160]  # [1, 128]
            ones32 = cst_pool.tile([P, 1], f32, name="ones32", tag="ones32")
            nc.vector.memset(ones32[:], 1.0)
            # x rows (AV stationary operand): resident for all of phase 2
            xn16 = [xn_pool.tile([P, DIN], f16, name=f"xn{c}", tag=f"xn{c}")
                    for c in range(NK)]

            # ---------------- phase 1: t-projection ----------------
            with tc.tile_pool(name="xs", bufs=16) as x_pool, \
                 tc.tile_pool(name="ws", bufs=8) as w_pool, \
                 tc.tile_pool(name="pps", bufs=5, space="PSUM") as proj_ps:
                # PE warmup during the initial DMA head: harmless matmuls on
                # the (memset, DMA-free) zero tile keep the HAM clock gate
                # from idling while the first x/A tiles stream in.
                wu = proj_ps.tile([P, 128], f32, name="wu", tag="wu", bufs=1)
                for r in range(48):
                    nc.tensor.matmul(wu[:], zeroT[:, 0:128], zeroT[:, 0:128],
                                     start=True, stop=True,
                                     skip_group_check=True)

                # tT = A^T @ x^T, all 2048 keys in two column halves.
                # d-outer loop with 2 concurrent PSUM chains -> each weight
                # slice (lhsT) is loaded once per 2 matmuls.
                # interleave the aT/xh0 loads d-wise so the first t-proj
                # accumulation chain can trickle-start as operand pairs land
                # instead of waiting for the whole 4MB group
                ats = []
                xhs_all = {}
                for d in range(ND):
                    at = w_pool.tile([P, DIN], f16, name=f"at{d}", tag="ws")
                    nc.sync.dma_start(at[:], aT_d[d * P:(d + 1) * P, :])
                    ats.append(at)
                    xh = x_pool.tile([P, 1024], f16, name=f"xh0_{d}", tag="xs")
                    nc.sync.dma_start(xh[:], xT_d[d * P:(d + 1) * P, 0:1024])
                    xhs_all[(0, d)] = xh
                for d in range(ND):
                    xh = x_pool.tile([P, 1024], f16, name=f"xh1_{d}", tag="xs")
                    nc.sync.dma_start(
                        xh[:], xT_d[d * P:(d + 1) * P, 1024:2048])
                    xhs_all[(1, d)] = xh
                # score rhs operand (x^T gathered q cols) prefetches right
                # behind the t-proj operands (own pool slots -> no WAR
                # serialization)
                for d in range(ND):
                    nc.sync.dma_start(xqs[d][:], xqT_d[d * P:(d + 1) * P, :])
                # attention stationary x rows stream in behind everything;
                # the tiny constants (needed only mid-phase-2) come last
                for c in range(NK):
                    nc.sync.dma_start(xn16[c][:], xn_d[c * P:(c + 1) * P, :])
                nc.sync.dma_start(onesT[:], ones_d[:])

                # two output chains in flight (4 PSUM tiles): during the
                # initial DMA ramp each landed (aT_d, xh_d) pair feeds 4
                # matmuls instead of 2, matching the delivery cadence so the
                # PE is not starved while the operands stream in
                for half in range(2):
                    xhs = [xhs_all[(half, d)] for d in range(ND)]
                    for o0 in range(0, NO, 2):
                        pos = [proj_ps.tile([P, 512], f32,
                                            name=f"pok{oi}_{kp}", tag="po")
                               for oi in range(2) for kp in range(2)]
                        for d in range(ND):
                            for oi in range(2):
                                for kp in range(2):
                                    nc.tensor.matmul(
                                        pos[oi * 2 + kp][:],
                                        ats[d][:, (o0 + oi) * P:
                                               (o0 + oi + 1) * P],
                                        xhs[d][:, kp * 512:(kp + 1) * 512],
                                        start=(d == 0), stop=(d == ND - 1))
                        for oi in range(2):
                            for kp in range(2):
                                nc.vector.tensor_copy(
                                    tT[o0 + oi][half][:, kp * 512:
                                                      kp * 512 + 512],
                                    pos[oi * 2 + kp][:])

            # ------- phase 2: attention + interleaved out-projection -------
            with tc.tile_pool(name="exq", bufs=5) as exp_pool, \
                 tc.tile_pool(name="dac", bufs=2) as dacc_pool, \
                 tc.tile_pool(name="obp", bufs=4) as out_pool, \
                 tc.tile_pool(name="sps", bufs=2, space="PSUM") as sT_ps, \
                 tc.tile_pool(name="cps", bufs=4, space="PSUM") as ctx_ps, \
                 tc.tile_pool(name="ops", bufs=2, space="PSUM") as out_ps:
                maskT = cst_pool.tile([P, 16 * P], f16, name="maskT",
                                      tag="maskT")
                nc.sync.dma_start(maskT[:], masks_d[:])
                wvs = []
                for d in range(ND):
                    wv = wv_pool.tile([P, DOUT], f16, name=f"wv{d}",
                                      tag=f"wv{d}")
                    nc.sync.dma_start(wv[:], wvT_d[d * P:(d + 1) * P, :])
                    wvs.append(wv)

                # per-pair tiles: ctx [128,256] per d (slot s -> column
                # half s%2), reciprocal [1,256]
                ctxP = {}
                recP = {}
                for p in range(NS // 2):
                    for d in range(ND):
                        ctxP[(d, p)] = ctx_pool.tile(
                            [P, 256], f16, name=f"ctx{d}_{p}",
                            tag=f"ctx{d}_{p}")
                    recP[p] = cst_pool.tile([1, 256], f16, name=f"rec{p}",
                                            tag=f"rec{p}")

                def do_outproj(p):
                    # outT[:, p*256:(p+1)*256] = (Wv ctx^T) * (1/denominator)
                    bc = out_ps.tile([P, 256], f32, name="bc", tag="poo")
                    nc.tensor.matmul(bc[:], ones_row, recP[p][:],
                                     start=True, stop=True)
                    bcs = out_pool.tile([P, 256], f32, name="bcs", tag="bcs",
                                        bufs=2)
                    nc.vector.tensor_copy(bcs[:], bc[:])
                    for o in range(NO):
                        po = out_ps.tile([P, 256], f32, name="poo", tag="poo")
                        for d in range(ND):
                            nc.tensor.matmul(
                                po[:],
                                wvs[d][:, o * P:(o + 1) * P],
                                ctxP[(d, p)][:],
                                start=(d == 0), stop=(d == ND - 1))
                        ob = out_pool.tile([P, 256], f32, name="ob", tag="ob",
                                           bufs=2)
                        nc.vector.tensor_mul(ob[:], po[:], bcs[:])
                        nc.sync.dma_start(
                            outT_d[o * P:(o + 1) * P, p * 256:(p + 1) * 256],
                            ob[:])

                daccP = [None]
                for s in range(NS):
                    q0 = s * P
                    ls = L[s]
                    cps = [ctx_ps.tile([P, 512], f32, name=f"cps{s}_{i}",
                                       tag="cps") for i in range(2)]
                    # denominator partials for the whole slot pair live in
                    # one [128,256] tile (slot -> column half) so the pair
                    # needs a single partition-reduce matmul + reciprocal
                    if s % 2 == 0:
                        daccP[0] = dacc_pool.tile([P, 256], f32,
                                                  name=f"dacc{s // 2}",
                                                  tag="dacc")
                    dacc = daccP[0][:, (s % 2) * P:(s % 2) * P + P]

                    def st_chunk(c):
                        st = sT_ps.tile([P, P], f32, name="st", tag="st")
                        for o in range(NO):
                            nc.tensor.matmul(
                                st[:],
                                tT[o][c // 8][:, (c % 8) * P:(c % 8 + 1) * P],
                                xqs[o][:, q0:q0 + P],
                                start=(o == 0), stop=(o == NO - 1))
                        et = exp_pool.tile([P, P], f16, name="et", tag="et")
                        nc.scalar.activation(et[:], st[:], EXP, scale=1.0 / 32.0)
                        if c >= ls - 2:
                            m = 2 * s + (c - (ls - 2))
                            et2 = exp_pool.tile([P, P], f16, name="et2",
                                                tag="et2")
                            nc.vector.tensor_mul(
                                et2[:], et[:], maskT[:, m * P:(m + 1) * P])
                            et = et2
                        return et

                    def av_chunk(c, et):
                        # softmax denominator partials accumulate on the
                        # (otherwise idle) DVE instead of spending PE
                        # matmuls; one fp32 ones-matmul per slot reduces over
                        # partitions afterwards
                        if c == 0:
                            nc.vector.tensor_copy(dacc, et[:])
                        else:
                            nc.vector.tensor_add(dacc, dacc, et[:])
                        for d in range(ND):
                            acc = cps[d // 4][:, (d % 4) * P:(d % 4) * P + P]
                            nc.tensor.matmul(
                                acc, xn16[c][:, d * P:(d + 1) * P], et[:],
                                start=False, stop=(c == ls - 1),
                                skip_group_check=True)

                    # software pipeline: score chains run 2 chunks ahead of
                    # the AV matmuls.  Zero the ctx banks with DVE memsets
                    # instead of dummy matmuls: with the data zeroed, a
                    # start=False matmul is correct for ANY has_written state
                    # (set bit -> accumulate onto 0; clear bit -> plain
                    # overwrite that sets the bit), and the PE spends nothing
                    # on initialization.
                    ets = {0: st_chunk(0)}
                    if ls > 1:
                        ets[1] = st_chunk(1)
                    for i in range(2):
                        nc.vector.memset(cps[i][:], 0.0)
                    # the previous pair's out-projection slots in here: its
                    # reciprocal (DVE, queued at the pair boundary) has
                    # drained behind the two score chains above, so the bc
                    # broadcast matmul never stalls the PE queue
                    if s >= 2 and s % 2 == 0:
                        do_outproj(s // 2 - 1)
                    for c in range(ls):
                        if c + 2 < ls:
                            ets[c + 2] = st_chunk(c + 2)
                        av_chunk(c, ets.pop(c))

                    # evacuate ctx accumulators into the per-pair tiles
                    # (frees the PSUM banks without waiting on the
                    # denominator chain).  Alternate evacuations onto the
                    # otherwise-idle scalar engine to unload the DVE.
                    for d in range(ND):
                        srcp = cps[d // 4][:, (d % 4) * P:(d % 4) * P + P]
                        dst = ctxP[(d, s // 2)][:, (s % 2) * P:(s % 2) * P + P]
                        if d % 2 == 0:
                            nc.scalar.copy(dst, srcp)
                        else:
                            nc.vector.tensor_copy(dst, srcp)
                    # at the pair boundary: partition-reduce both slots'
                    # denominator partials (their DVE chains finished long
                    # ago, so the PE never stalls here), then reciprocal
                    # into the pair's [1,256] tile.  The out-projection
                    # itself is deferred into the next slot's pipeline.
                    if s % 2 == 1:
                        # dsum lives in the out_ps rotation: its reciprocal
                        # (scalar engine, short queue) drains while the next
                        # slot's score chains run, so neither the next po
                        # chain nor st chain ever waits on it
                        dsum = out_ps.tile([1, 256], f32, name=f"dsum{s // 2}",
                                           tag="poo")
                        nc.tensor.matmul(dsum[:], ones32[:], daccP[0][:],
                                         start=True, stop=True)
                        with nc.allow_low_precision(
                                reason="fp16 recip feeds fp16 bcast"):
                            nc.vector.reciprocal(recP[s // 2][:], dsum[:])
                do_outproj(NS // 2 - 1)

    nc.compile()
    return nc


def _get_nc():
    if "nc" not in _NC_CACHE:
        _NC_CACHE["nc"] = _build_nc()
    return _NC_CACHE["nc"]


def _make_masks(h):
    """[128, 16*128] mask tiles: slot s, last-2 chunks; 1.0 where key
    128c+p <= query 128*BLK[h][s]+j (all-ones / triangular / all-zero)."""
    mk = np.zeros((P, 16 * P), dtype=np.float16)
    p = np.arange(P)[:, None]
    j = np.arange(P)[None, :]
    for s in range(NS):
        b = BLK[h][s]
        for m in range(2):
            c = L[s] - 2 + m
            mk[:, (2 * s + m) * P:(2 * s + m + 1) * P] = (
                (P * c + p) <= (P * b + j)).astype(np.float16)
    return mk


def kernel(x, W_q, W_k, W_v):
    from concourse.bass_utils import run_bass_kernel_spmd

    x = np.asarray(x, dtype=np.float32)
    x16 = x.astype(np.float16)
    wq = np.asarray(W_q, dtype=np.float32)
    wk = np.asarray(W_k, dtype=np.float32)
    # A^T = W_k^T W_q  (scores = x_q (W_q^T W_k) x_k^T; tT = (A^T)^T x^T)
    aT = np.ascontiguousarray((wk.T @ wq).astype(np.float16))
    wvT = np.ascontiguousarray(np.asarray(W_v, dtype=np.float32).T
                               .astype(np.float16))

    ones = np.zeros((P, 160), dtype=np.float16)
    ones[:, 0] = 1.0
    ones[0, 32:160] = 1.0
    masks_h = [_make_masks(0), _make_masks(1)]

    in_maps = []
    for b in range(B):
        xTb = np.ascontiguousarray(x16[b].T)
        for h in range(2):
            qcols = np.concatenate(
                [np.arange(g * P, (g + 1) * P) for g in BLK[h]])
            in_maps.append(dict(
                xqT=np.ascontiguousarray(xTb[:, qcols]),
                xT=xTb,
                xn=np.ascontiguousarray(x16[b]),
                aT=aT, wvT=wvT,
                masks=masks_h[h],
                ones=ones,
            ))

    nc = _get_nc()
    res = run_bass_kernel_spmd(nc, in_maps, core_ids=list(range(NCORES)),
                               trace=bool(os.environ.get("KERNEL_TRACE")))
    if os.environ.get("KERNEL_TRACE"):
        _NC_CACHE["last_results"] = res

    out = np.empty((B, S, DOUT), dtype=np.float32)
    for b in range(B):
        for h in range(2):
            oT = res.results[b * 2 + h]["outT"]
            for s2, g in enumerate(BLK[h]):
                out[b, g * P:(g + 1) * P, :] = \
                    oT[:, s2 * P:(s2 + 1) * P].T
    return out
